# revision 38
# baseline (speedup 1.0000x reference)
"""Tensor-parallel attention kernel for 8 Trainium2 NeuronCores.

Reference computation (S=2048, B=2, H=2048, NH=16 heads, HD=128):
    q = x @ wq.T ; k = x @ wk.T ; v = x @ wv.T          (x: [S, B, H])
    per (b, head): out = softmax(q k^T / sqrt(HD)) v
    return concat_heads(out) @ wo.T                      ([S, B, H])

Sharding: tensor-parallel over heads (column-parallel wq/wk/wv shards). Core r
owns heads {2r, 2r+1}. The cross-core combine happens BEFORE the output
projection via AllToAll of bf16 attention outputs; each core then applies the
full wo to its 512-token slice.

Schedule (all tokens b-major t = b*S + s):
  phase 1 (per b): qT/kT [256 feat, 2048 tok] = w.T @ x; v [tok, 256] natural
  phase 2, h-outer: all (b, qt) attention units for head 0, then AllToAll #0
      (1 MB) fires while head 1 computes; AllToAll #1 after head 1.
      Softmax denominators: VectorE accumulates the exp tiles (bf16) and a
      single ones-matmul per unit does the partition reduction (replaces 16
      sum-matmuls per unit on the PE).
  phase 3 split by kt parity: even-kt (head-0 senders, delivered by A2A#0)
      partial products run DURING A2A#1; odd-kt + partial add after. Keeps
      the PE busy and HAM-warm through the collective.

DMA: hosts pre-shuffles x/wq/wk/wv/wo into SBUF-tile-order DRAM layouts so
each load is one large DMA with >=1KB contiguous runs (DMA issue on an engine
queue costs ~0.7us each; the baseline spent ~120us of queue time on issues).
"""

import numpy as np

S, B, H = 2048, 2, 2048
NH, HD = 16, 128
N_CORES = 8
HPC = NH // N_CORES          # heads per core (2)
FPC = HPC * HD               # features per core (256)
NT = S * B                   # tokens (4096)
SCALE = HD ** -0.5
KT = H // 128                # contraction tiles (16)
NB = 512                     # token block width in phase 1
XW = KT * NB                 # x big-tile width (8192)
QT = 512                     # q-tile width in phase 2
EXPW = 1024                  # exp batch width (2 key-blocks per ACT op)
JB = S // 128                # key blocks per (b, h) (16)


def _build():
    import concourse.mybir as mybir
    import concourse.tile as tile
    from concourse import bacc

    F32 = mybir.dt.float32
    BF16 = mybir.dt.bfloat16
    Exp = mybir.ActivationFunctionType.Exp

    nc = bacc.Bacc(None, target_bir_lowering=False, num_devices=N_CORES)

    # Pre-shuffled inputs (see make_in_maps):
    #   xS[nb*128+p, kt*NB+t] = x_bf16[feature kt*128+p, token nb*NB+t]
    #   w*S[p, kt*FPC+f]      = w[sl].T[kt*128+p, f]
    #   woS[(nt*2+par)*128+p, j*512+t] = wo.T[(2j+par)*128+p, nt*512+t]
    xS = nc.dram_tensor("xS", [8 * 128, XW], BF16, kind="ExternalInput")
    wqS = nc.dram_tensor("wqS", [128, KT * FPC], BF16, kind="ExternalInput")
    wkS = nc.dram_tensor("wkS", [128, KT * FPC], BF16, kind="ExternalInput")
    wvS = nc.dram_tensor("wvS", [128, KT * FPC], BF16, kind="ExternalInput")
    woS = nc.dram_tensor("woS", [8 * 128, 8 * 512], BF16, kind="ExternalInput")
    out = nc.dram_tensor("out", [NT // N_CORES, H], F32, kind="ExternalOutput")

    from contextlib import ExitStack

    with tile.TileContext(nc) as tc, ExitStack() as ctx:
        pool = lambda **kw: ctx.enter_context(tc.tile_pool(**kw))
        qk_res = pool(name="qk_res", bufs=1)
        v_res = pool(name="v_res", bufs=32)
        const = pool(name="const", bufs=1)
        x0_p = pool(name="x0_p", bufs=2)
        x_p = pool(name="x_p", bufs=2)
        w_p1 = pool(name="w_p1", bufs=1)
        wo_p = pool(name="wo_p", bufs=6)
        p_p2 = pool(name="p_p2", bufs=3)
        acc_p = pool(name="acc_p", bufs=1)
        r_p2 = pool(name="r_p2", bufs=1)
        # 3 bufs: o_send DMA completion (DRAM write) can lag badly when the
        # fabric/HBM is congested; without slack here the pv PSUM pool
        # back-pressures and stalls the PE mid-attention.
        ost_p = pool(name="ost_p", bufs=3)
        or_p = pool(name="or_p", bufs=1)
        part_p = pool(name="part_p", bufs=16)
        ev_p3 = pool(name="ev_p3", bufs=1)
        ps_qk = pool(name="ps_qk", bufs=2, space="PSUM")
        ps_sc = pool(name="ps_sc", bufs=2, space="PSUM")
        ps_pv = pool(name="ps_pv", bufs=2, space="PSUM")
        dram = pool(name="dram", bufs=1, space="DRAM")

        ones_f = const.tile([128, 128], F32)
        nc.vector.memset(ones_f[:], 1.0)
        ones = const.tile([128, 128], BF16)
        nc.vector.tensor_copy(ones[:], ones_f[:])

        qhat = [qk_res.tile([128, NT], BF16, tag=f"q{m}", name=f"qhat{m}")
                for m in range(2)]
        khat = [qk_res.tile([128, NT], BF16, tag=f"k{m}", name=f"khat{m}")
                for m in range(2)]
        vsb = [v_res.tile([128, FPC], BF16, tag="v", name=f"vsb{i}")
               for i in range(NT // 128)]
        o_send = [dram.tile([8 * 128, QT], BF16, name=f"o_send{h}")
                  for h in range(2)]
        o_recv = [dram.tile([8 * 128, QT], BF16, name=f"o_recv{h}")
                  for h in range(2)]

        # ---- input loads -------------------------------------------------
        # weights on the scalar queue; x on the sync queue. All single big
        # DMAs with long contiguous runs thanks to the host pre-shuffle.
        # First q/k group needs wq kt0.. + x0 kt0.. — split the leading loads
        # into halves/quarters across idle queues so the first matmul can
        # start as early as possible after the ~9.5us framework preamble.
        # Startup is DMA-bandwidth-bound: ~5MB (wq+x0+wk+wv) feeds the first
        # ~26us of matmuls. Spread it over three queues (sync, scalar,
        # gpsimd SWDGE) roughly in consumption order.
        wq_all = w_p1.tile([128, KT * FPC], BF16, tag="wq", name="wq_all")
        nc.scalar.dma_start(wq_all[:, 0:KT * FPC // 2], wqS[:, 0:KT * FPC // 2])
        x0a = x0_p.tile([128, XW // 2], BF16, tag="x0a", name="x0a")
        nc.sync.dma_start(x0a[:, 0:XW // 4], xS[0:128, 0:XW // 4])
        nc.scalar.dma_start(wq_all[:, KT * FPC // 2:], wqS[:, KT * FPC // 2:])
        nc.sync.dma_start(x0a[:, XW // 4:], xS[0:128, XW // 4:XW // 2])
        x0b = x0_p.tile([128, XW // 2], BF16, tag="x0b", name="x0b")
        nc.gpsimd.dma_start(x0b[:, 0:XW // 4], xS[0:128, XW // 2:3 * XW // 4])
        nc.gpsimd.dma_start(x0b[:, XW // 4:], xS[0:128, 3 * XW // 4:XW])
        wk_all = w_p1.tile([128, KT * FPC], BF16, tag="wk", name="wk_all")
        nc.scalar.dma_start(wk_all[:, 0:KT * FPC // 2], wkS[:, 0:KT * FPC // 2])
        nc.sync.dma_start(wk_all[:, KT * FPC // 2:], wkS[:, KT * FPC // 2:])
        wv_all = w_p1.tile([128, KT * FPC], BF16, tag="wv", name="wv_all")
        nc.gpsimd.dma_start(wv_all[:], wvS[:, :])

        wq_t = [wq_all[:, kt * FPC:(kt + 1) * FPC] for kt in range(KT)]
        wk_t = [wk_all[:, kt * FPC:(kt + 1) * FPC] for kt in range(KT)]
        wv_t = [wv_all[:, kt * FPC:(kt + 1) * FPC] for kt in range(KT)]

        x_big = {}

        def load_x(nb_list):
            for nb in nb_list:
                t = x_p.tile([128, XW], BF16, tag="x", name=f"x{nb}")
                nc.sync.dma_start(t[:], xS[nb * 128:(nb + 1) * 128, :])
                x_big[nb] = t

        def xt(nb, kt):
            if nb == 0:
                src = x0a if kt < 8 else x0b
                k = kt if kt < 8 else kt - 8
                return src[:, k * NB:(k + 1) * NB]
            return x_big[nb][:, kt * NB:(kt + 1) * NB]

        # wo parity half-tiles: wo_t[nt][par][half][:, jj*512:(jj+1)*512] is
        # the woT tile for contraction block kt=2*(half*4+jj)+par, output
        # columns nt*512... Loaded in 4KB/partition chunks through a 4-buffer
        # pool so phase-3 streaming never stalls on a 1MB transfer.
        wo_t = [[[None, None], [None, None]] for _ in range(4)]
        for par in range(2):
            for nt in range(4):
                r0 = (nt * 2 + par) * 128
                for half in range(2):
                    t = wo_p.tile([128, 4 * 512], BF16, tag="wo",
                                  name=f"wo{nt}_{par}_{half}")
                    # par1 (3b-era) tiles stream during phase 3; alternate
                    # queues to double the load bandwidth there
                    eng = nc.scalar if (par == 0 or half == 0) else nc.gpsimd
                    eng.dma_start(
                        t[:], woS[r0:r0 + 128, half * 2048:(half + 1) * 2048]
                    )
                    wo_t[nt][par][half] = t

        # ---- phase 1: qT/kT/v projections --------------------------------
        def phase1(b):
            for nb in range(b * S // NB, (b + 1) * S // NB):
                for dest, wt in ((qhat, wq_t), (khat, wk_t)):
                    for m in range(2):
                        ps = ps_qk.tile([128, NB], F32, tag="qk")
                        for kt in range(KT):
                            nc.tensor.matmul(
                                ps[:],
                                wt[kt][:, m * 128:(m + 1) * 128],
                                xt(nb, kt),
                                start=(kt == 0),
                                stop=(kt == KT - 1),
                            )
                        nc.vector.tensor_copy(
                            dest[m][:, nb * NB:(nb + 1) * NB], ps[:]
                        )
                for sub in range(NB // 128):
                    ps = ps_qk.tile([128, FPC], F32, tag="qk")
                    for kt in range(KT):
                        nc.tensor.matmul(
                            ps[:],
                            xt(nb, kt)[:, sub * 128:(sub + 1) * 128],
                            wv_t[kt][:],
                            start=(kt == 0),
                            stop=(kt == KT - 1),
                        )
                    nc.vector.tensor_copy(vsb[nb * 4 + sub][:], ps[:])

        # ---- phase 2: attention for one (b, h, qt) unit ------------------
        def attention(b, h, qt):
            q_bh = qhat[h][:, b * S + qt * QT: b * S + (qt + 1) * QT]
            pv_ps = ps_pv.tile([128, QT], F32, tag="pv")
            acc = acc_p.tile([128, QT], BF16, tag="acc")
            for g in range(JB // 2):
                sc_ps = ps_sc.tile([128, EXPW], F32, tag="sc")
                pT = p_p2.tile([128, EXPW], BF16, tag="p")
                for i in range(2):
                    jb = g * 2 + i
                    nc.tensor.matmul(
                        sc_ps[:, i * QT:(i + 1) * QT],
                        khat[h][:, b * S + jb * 128: b * S + (jb + 1) * 128],
                        q_bh,
                        start=True,
                        stop=True,
                    )
                nc.scalar.activation(pT[:], sc_ps[:], Exp, scale=SCALE)
                if g == 0:
                    nc.vector.tensor_add(acc[:], pT[:, 0:QT], pT[:, QT:EXPW])
                else:
                    nc.vector.tensor_add(acc[:], acc[:], pT[:, 0:QT])
                    nc.vector.tensor_add(acc[:], acc[:], pT[:, QT:EXPW])
                for i in range(2):
                    jb = g * 2 + i
                    nc.tensor.matmul(
                        pv_ps[:],
                        vsb[b * JB + jb][:, h * 128:(h + 1) * 128],
                        pT[:, i * QT:(i + 1) * QT],
                        start=(jb == 0),
                        stop=(jb == JB - 1),
                    )
            sum_ps = ps_qk.tile([128, QT], F32, tag="qk")
            nc.tensor.matmul(sum_ps[:], ones[:], acc[:], start=True, stop=True)
            recip = r_p2.tile([128, QT], F32, tag="r")
            nc.vector.reciprocal_approx_fast(recip[:], sum_ps[:])
            ostg = ost_p.tile([128, QT], BF16, tag="ost")
            nc.vector.tensor_mul(ostg[:], pv_ps[:], recip[:])
            c = b * (S // QT) + qt
            nc.sync.dma_start(o_send[h][c * 128:(c + 1) * 128, :], ostg[:])

        # ---- main schedule ----------------------------------------------
        def a2a(h):
            # gpsimd reaches the trigger early and fires as soon as the
            # o_send[h] writes land; compute engines keep running.
            nc.gpsimd.collective_compute(
                "AllToAll",
                mybir.AluOpType.bypass,
                replica_groups=[list(range(N_CORES))],
                ins=[o_send[h][:].opt()],
                outs=[o_recv[h][:].opt()],
            )

        def load_or(h):
            # On the gpsimd queue, which is idle apart from the collective
            # triggers: the load starts the instant the A2A completes, and
            # never head-of-line-blocks a busy queue (a sync-queue DMA
            # waiting on A2A completion ahead of pending o_send writes would
            # back-pressure the attention pipeline via the pv PSUM pool).
            # Four chunks so phase 3 can start on the first senders' blocks
            # while the rest stream in.
            # or1 (h=1) additionally stripes across the sync queue — idle by
            # then, with only the out-writes behind it. or0 must NOT touch
            # sync: h=1 o_send writes are still pending there.
            engines = [nc.gpsimd] if h == 0 else [nc.gpsimd, nc.sync]
            t = or_p.tile([128, 8 * 512], BF16, tag=f"or{h}", name=f"or{h}")
            for ch in range(4):
                engines[ch % len(engines)].dma_start(
                    t[:, ch * 1024:(ch + 1) * 1024].rearrange(
                        "p (j t) -> p j t", j=2),
                    o_recv[h][ch * 256:(ch + 1) * 256, :].rearrange(
                        "(j p) t -> p j t", p=128),
                )
            return t

        or_big = [None, None]
        load_x([1, 2, 3])
        phase1(0)
        load_x([4, 5, 6, 7])
        for qt in range(S // QT):
            attention(0, 0, qt)
        phase1(1)
        for qt in range(S // QT):
            attention(1, 0, qt)
        a2a(0)  # fires now; transfers while h=1 computes
        or_big[0] = load_or(0)
        for qt in range(S // QT):
            attention(0, 1, qt)
        for qt in range(S // QT):
            attention(1, 1, qt)
        a2a(1)
        or_big[1] = load_or(1)

        # ---- phase 3: out[tok_slice, :] = o.T @ wo.T ---------------------
        # 3a: even-kt partials (h=0 data, available after A2A#0) run during
        # A2A#1; 3b: odd-kt + partial add after A2A#1 lands.
        parts = {}
        for nt in range(4):
            for tb in range(4):
                ps = ps_pv.tile([128, 512], F32, tag="pv")
                for j in range(8):
                    nc.tensor.matmul(
                        ps[:],
                        or_big[0][:, j * 512 + tb * 128: j * 512 + (tb + 1) * 128],
                        wo_t[nt][0][j // 4][:, (j % 4) * 512:(j % 4 + 1) * 512],
                        start=(j == 0),
                        stop=(j == 7),
                    )
                part = part_p.tile([128, 512], BF16, tag="part")
                nc.vector.tensor_copy(part[:], ps[:])
                parts[nt, tb] = part
        for nt in range(4):
            for tb in range(4):
                ps = ps_pv.tile([128, 512], F32, tag="pv")
                for j in range(8):
                    nc.tensor.matmul(
                        ps[:],
                        or_big[1][:, j * 512 + tb * 128: j * 512 + (tb + 1) * 128],
                        wo_t[nt][1][j // 4][:, (j % 4) * 512:(j % 4 + 1) * 512],
                        start=(j == 0),
                        stop=(j == 7),
                    )
                ev = ev_p3.tile([128, 512], F32, tag="ev")
                nc.vector.tensor_add(ev[:], ps[:], parts[nt, tb][:])
                nc.sync.dma_start(
                    out[tb * 128:(tb + 1) * 128, nt * 512:(nt + 1) * 512],
                    ev[:],
                )
    nc.compile()
    return nc


_NC_CACHE = None


def _get_nc():
    global _NC_CACHE
    if _NC_CACHE is None:
        _NC_CACHE = _build()
    return _NC_CACHE


def make_in_maps(x, wq, wk, wv, wo):
    import ml_dtypes

    bf = ml_dtypes.bfloat16
    x = np.asarray(x, dtype=np.float32)
    # tokens b-major: t = b*S + s
    xT = np.ascontiguousarray(x.transpose(2, 1, 0).reshape(H, NT))
    xS = np.ascontiguousarray(
        xT.reshape(KT, 128, 8, NB).transpose(2, 1, 0, 3).reshape(8 * 128, XW)
    ).astype(bf)
    woT = np.asarray(wo, dtype=np.float32).T  # [f_in, f_out]
    woS = np.ascontiguousarray(
        woT.reshape(8, 2, 128, 4, 512).transpose(3, 1, 2, 0, 4).reshape(
            8 * 128, 8 * 512)
    ).astype(bf)

    def wshuf(w, r):
        sl = slice(r * FPC, (r + 1) * FPC)
        wT = np.asarray(w, dtype=np.float32)[sl, :].T  # [H, FPC]
        return np.ascontiguousarray(
            wT.reshape(KT, 128, FPC).transpose(1, 0, 2).reshape(128, KT * FPC)
        ).astype(bf)

    in_maps = []
    for r in range(N_CORES):
        in_maps.append(
            {
                "xS": xS,
                "wqS": wshuf(wq, r),
                "wkS": wshuf(wk, r),
                "wvS": wshuf(wv, r),
                "woS": woS,
            }
        )
    return in_maps


def assemble_out(results):
    out_bs = np.concatenate([results[r]["out"] for r in range(N_CORES)], axis=0)
    return np.ascontiguousarray(out_bs.reshape(B, S, H).transpose(1, 0, 2))


def kernel(x, wq, wk, wv, wo):
    from concourse.bass_utils import run_bass_kernel_spmd

    in_maps = make_in_maps(x, wq, wk, wv, wo)
    res = run_bass_kernel_spmd(_get_nc(), in_maps, list(range(N_CORES)))
    return assemble_out(res.results)


# revision 39
# speedup vs baseline: 1.0497x; 1.0497x over previous
"""Tensor-parallel attention kernel for 8 Trainium2 NeuronCores.

Reference computation (S=2048, B=2, H=2048, NH=16 heads, HD=128):
    q = x @ wq.T ; k = x @ wk.T ; v = x @ wv.T          (x: [S, B, H])
    per (b, head): out = softmax(q k^T / sqrt(HD)) v
    return concat_heads(out) @ wo.T                      ([S, B, H])

Sharding: tensor-parallel over heads (column-parallel wq/wk/wv shards). Core r
owns heads {2r, 2r+1}. The cross-core combine happens BEFORE the output
projection via AllToAll of bf16 attention outputs; each core then applies the
full wo to its 512-token slice.

Schedule (all tokens b-major t = b*S + s):
  phase 1 (per b): qT/kT [256 feat, 2048 tok] = w.T @ x; v [tok, 256] natural
  phase 2, h-outer: all (b, qt) attention units for head 0, then AllToAll #0
      (1 MB) fires while head 1 computes; AllToAll #1 after head 1.
      Softmax denominators: VectorE accumulates the exp tiles (bf16) and a
      single ones-matmul per unit does the partition reduction (replaces 16
      sum-matmuls per unit on the PE).
  phase 3 split by kt parity: even-kt (head-0 senders, delivered by A2A#0)
      partial products run DURING A2A#1; odd-kt + partial add after. Keeps
      the PE busy and HAM-warm through the collective.

DMA: hosts pre-shuffles x/wq/wk/wv/wo into SBUF-tile-order DRAM layouts so
each load is one large DMA with >=1KB contiguous runs (DMA issue on an engine
queue costs ~0.7us each; the baseline spent ~120us of queue time on issues).
"""

import numpy as np

S, B, H = 2048, 2, 2048
NH, HD = 16, 128
N_CORES = 8
HPC = NH // N_CORES          # heads per core (2)
FPC = HPC * HD               # features per core (256)
NT = S * B                   # tokens (4096)
SCALE = HD ** -0.5
KT = H // 128                # contraction tiles (16)
NB = 512                     # token block width in phase 1
XW = KT * NB                 # x big-tile width (8192)
QT = 512                     # q-tile width in phase 2
EXPW = 1024                  # exp batch width (2 key-blocks per ACT op)
JB = S // 128                # key blocks per (b, h) (16)


def _build():
    import concourse.mybir as mybir
    import concourse.tile as tile
    from concourse import bacc

    F32 = mybir.dt.float32
    BF16 = mybir.dt.bfloat16
    Exp = mybir.ActivationFunctionType.Exp

    nc = bacc.Bacc(None, target_bir_lowering=False, num_devices=N_CORES)

    # Pre-shuffled inputs (see make_in_maps):
    #   xS[nb*128+p, kt*NB+t] = x_bf16[feature kt*128+p, token nb*NB+t]
    #   w*S[p, kt*FPC+f]      = w[sl].T[kt*128+p, f]
    #   woS[(nt*2+par)*128+p, j*512+t] = wo.T[(2j+par)*128+p, nt*512+t]
    xS = nc.dram_tensor("xS", [8 * 128, XW], BF16, kind="ExternalInput")
    wqS = nc.dram_tensor("wqS", [128, KT * FPC], BF16, kind="ExternalInput")
    wkS = nc.dram_tensor("wkS", [128, KT * FPC], BF16, kind="ExternalInput")
    wvS = nc.dram_tensor("wvS", [128, KT * FPC], BF16, kind="ExternalInput")
    woS = nc.dram_tensor("woS", [8 * 128, 8 * 512], BF16, kind="ExternalInput")
    out = nc.dram_tensor("out", [NT // N_CORES, H], F32, kind="ExternalOutput")

    from contextlib import ExitStack

    with tile.TileContext(nc) as tc, ExitStack() as ctx:
        pool = lambda **kw: ctx.enter_context(tc.tile_pool(**kw))
        qk_res = pool(name="qk_res", bufs=1)
        v_res = pool(name="v_res", bufs=32)
        const = pool(name="const", bufs=1)
        x0_p = pool(name="x0_p", bufs=2)
        x_p = pool(name="x_p", bufs=2)
        w_p1 = pool(name="w_p1", bufs=1)
        wo_p = pool(name="wo_p", bufs=6)
        p_p2 = pool(name="p_p2", bufs=3)
        acc_p = pool(name="acc_p", bufs=1)
        r_p2 = pool(name="r_p2", bufs=1)
        # 3 bufs: o_send DMA completion (DRAM write) can lag badly when the
        # fabric/HBM is congested; without slack here the pv PSUM pool
        # back-pressures and stalls the PE mid-attention.
        ost_p = pool(name="ost_p", bufs=3)
        or_p = pool(name="or_p", bufs=1)
        part_p = pool(name="part_p", bufs=16)
        ev_p3 = pool(name="ev_p3", bufs=1)
        ps_qk = pool(name="ps_qk", bufs=2, space="PSUM")
        ps_sc = pool(name="ps_sc", bufs=2, space="PSUM")
        ps_pv = pool(name="ps_pv", bufs=2, space="PSUM")
        dram = pool(name="dram", bufs=1, space="DRAM")

        ones_f = const.tile([128, 128], F32)
        nc.vector.memset(ones_f[:], 1.0)
        ones = const.tile([128, 128], BF16)
        nc.vector.tensor_copy(ones[:], ones_f[:])

        qhat = [qk_res.tile([128, NT], BF16, tag=f"q{m}", name=f"qhat{m}")
                for m in range(2)]
        khat = [qk_res.tile([128, NT], BF16, tag=f"k{m}", name=f"khat{m}")
                for m in range(2)]
        vsb = [v_res.tile([128, FPC], BF16, tag="v", name=f"vsb{i}")
               for i in range(NT // 128)]
        o_send = [dram.tile([8 * 128, QT], BF16, name=f"o_send{h}")
                  for h in range(2)]
        o_recv = [dram.tile([8 * 128, QT], BF16, name=f"o_recv{h}")
                  for h in range(2)]

        # ---- input loads -------------------------------------------------
        # weights on the scalar queue; x on the sync queue. All single big
        # DMAs with long contiguous runs thanks to the host pre-shuffle.
        # First q/k group needs wq kt0.. + x0 kt0.. — split the leading loads
        # into halves/quarters across idle queues so the first matmul can
        # start as early as possible after the ~9.5us framework preamble.
        # Startup is DMA-bandwidth-bound: ~5MB (wq+x0+wk+wv) feeds the first
        # ~26us of matmuls. Spread it over three queues (sync, scalar,
        # gpsimd SWDGE) roughly in consumption order.
        wq_all = w_p1.tile([128, KT * FPC], BF16, tag="wq", name="wq_all")
        nc.scalar.dma_start(wq_all[:, 0:KT * FPC // 2], wqS[:, 0:KT * FPC // 2])
        x0a = x0_p.tile([128, XW // 2], BF16, tag="x0a", name="x0a")
        nc.sync.dma_start(x0a[:, 0:XW // 4], xS[0:128, 0:XW // 4])
        nc.scalar.dma_start(wq_all[:, KT * FPC // 2:], wqS[:, KT * FPC // 2:])
        nc.sync.dma_start(x0a[:, XW // 4:], xS[0:128, XW // 4:XW // 2])
        x0b = x0_p.tile([128, XW // 2], BF16, tag="x0b", name="x0b")
        nc.gpsimd.dma_start(x0b[:, 0:XW // 4], xS[0:128, XW // 2:3 * XW // 4])
        nc.gpsimd.dma_start(x0b[:, XW // 4:], xS[0:128, 3 * XW // 4:XW])
        wk_all = w_p1.tile([128, KT * FPC], BF16, tag="wk", name="wk_all")
        nc.scalar.dma_start(wk_all[:, 0:KT * FPC // 2], wkS[:, 0:KT * FPC // 2])
        nc.sync.dma_start(wk_all[:, KT * FPC // 2:], wkS[:, KT * FPC // 2:])
        wv_all = w_p1.tile([128, KT * FPC], BF16, tag="wv", name="wv_all")
        nc.gpsimd.dma_start(wv_all[:], wvS[:, :])

        wq_t = [wq_all[:, kt * FPC:(kt + 1) * FPC] for kt in range(KT)]
        wk_t = [wk_all[:, kt * FPC:(kt + 1) * FPC] for kt in range(KT)]
        wv_t = [wv_all[:, kt * FPC:(kt + 1) * FPC] for kt in range(KT)]

        x_big = {}

        def load_x(nb_list):
            for nb in nb_list:
                t = x_p.tile([128, XW], BF16, tag="x", name=f"x{nb}")
                nc.sync.dma_start(t[:], xS[nb * 128:(nb + 1) * 128, :])
                x_big[nb] = t

        def xt(nb, kt):
            if nb == 0:
                src = x0a if kt < 8 else x0b
                k = kt if kt < 8 else kt - 8
                return src[:, k * NB:(k + 1) * NB]
            return x_big[nb][:, kt * NB:(kt + 1) * NB]

        # wo parity half-tiles: wo_t[nt][par][half][:, jj*512:(jj+1)*512] is
        # the woT tile for contraction block kt=2*(half*4+jj)+par, output
        # columns nt*512... Loaded in 4KB/partition chunks through a 4-buffer
        # pool so phase-3 streaming never stalls on a 1MB transfer.
        wo_t = [[[None, None], [None, None]] for _ in range(4)]
        for par in range(2):
            for nt in range(4):
                r0 = (nt * 2 + par) * 128
                for half in range(2):
                    t = wo_p.tile([128, 4 * 512], BF16, tag="wo",
                                  name=f"wo{nt}_{par}_{half}")
                    # scalar (hardware DGE) only: gpsimd SWDGE measured slower
                    # and starves the 3b stream
                    nc.scalar.dma_start(
                        t[:], woS[r0:r0 + 128, half * 2048:(half + 1) * 2048]
                    )
                    wo_t[nt][par][half] = t

        # ---- phase 1: qT/kT/v projections --------------------------------
        def phase1(b):
            for nb in range(b * S // NB, (b + 1) * S // NB):
                for dest, wt in ((qhat, wq_t), (khat, wk_t)):
                    for m in range(2):
                        ps = ps_qk.tile([128, NB], F32, tag="qk")
                        for kt in range(KT):
                            nc.tensor.matmul(
                                ps[:],
                                wt[kt][:, m * 128:(m + 1) * 128],
                                xt(nb, kt),
                                start=(kt == 0),
                                stop=(kt == KT - 1),
                            )
                        nc.vector.tensor_copy(
                            dest[m][:, nb * NB:(nb + 1) * NB], ps[:]
                        )
                for sub in range(NB // 128):
                    ps = ps_qk.tile([128, FPC], F32, tag="qk")
                    for kt in range(KT):
                        nc.tensor.matmul(
                            ps[:],
                            xt(nb, kt)[:, sub * 128:(sub + 1) * 128],
                            wv_t[kt][:],
                            start=(kt == 0),
                            stop=(kt == KT - 1),
                        )
                    nc.vector.tensor_copy(vsb[nb * 4 + sub][:], ps[:])

        # ---- phase 2: attention for one (b, h, qt) unit ------------------
        def attention(b, h, qt):
            q_bh = qhat[h][:, b * S + qt * QT: b * S + (qt + 1) * QT]
            pv_ps = ps_pv.tile([128, QT], F32, tag="pv")
            acc = acc_p.tile([128, QT], BF16, tag="acc")
            for g in range(JB // 2):
                sc_ps = ps_sc.tile([128, EXPW], F32, tag="sc")
                pT = p_p2.tile([128, EXPW], BF16, tag="p")
                for i in range(2):
                    jb = g * 2 + i
                    nc.tensor.matmul(
                        sc_ps[:, i * QT:(i + 1) * QT],
                        khat[h][:, b * S + jb * 128: b * S + (jb + 1) * 128],
                        q_bh,
                        start=True,
                        stop=True,
                    )
                nc.scalar.activation(pT[:], sc_ps[:], Exp, scale=SCALE)
                if g == 0:
                    nc.vector.tensor_add(acc[:], pT[:, 0:QT], pT[:, QT:EXPW])
                else:
                    nc.vector.tensor_add(acc[:], acc[:], pT[:, 0:QT])
                    nc.vector.tensor_add(acc[:], acc[:], pT[:, QT:EXPW])
                for i in range(2):
                    jb = g * 2 + i
                    nc.tensor.matmul(
                        pv_ps[:],
                        vsb[b * JB + jb][:, h * 128:(h + 1) * 128],
                        pT[:, i * QT:(i + 1) * QT],
                        start=(jb == 0),
                        stop=(jb == JB - 1),
                    )
            sum_ps = ps_qk.tile([128, QT], F32, tag="qk")
            nc.tensor.matmul(sum_ps[:], ones[:], acc[:], start=True, stop=True)
            recip = r_p2.tile([128, QT], F32, tag="r")
            nc.vector.reciprocal_approx_fast(recip[:], sum_ps[:])
            ostg = ost_p.tile([128, QT], BF16, tag="ost")
            nc.vector.tensor_mul(ostg[:], pv_ps[:], recip[:])
            c = b * (S // QT) + qt
            nc.sync.dma_start(o_send[h][c * 128:(c + 1) * 128, :], ostg[:])

        # ---- main schedule ----------------------------------------------
        def a2a(h):
            # gpsimd reaches the trigger early and fires as soon as the
            # o_send[h] writes land; compute engines keep running.
            nc.gpsimd.collective_compute(
                "AllToAll",
                mybir.AluOpType.bypass,
                replica_groups=[list(range(N_CORES))],
                ins=[o_send[h][:].opt()],
                outs=[o_recv[h][:].opt()],
            )

        def load_or(h):
            # On the gpsimd queue, which is idle apart from the collective
            # triggers: the load starts the instant the A2A completes, and
            # never head-of-line-blocks a busy queue (a sync-queue DMA
            # waiting on A2A completion ahead of pending o_send writes would
            # back-pressure the attention pipeline via the pv PSUM pool).
            # Four chunks so phase 3 can start on the first senders' blocks
            # while the rest stream in.
            # or1 (h=1) additionally stripes across the sync queue — idle by
            # then, with only the out-writes behind it. or0 must NOT touch
            # sync: h=1 o_send writes are still pending there.
            engines = [nc.gpsimd] if h == 0 else [nc.gpsimd, nc.sync]
            t = or_p.tile([128, 8 * 512], BF16, tag=f"or{h}", name=f"or{h}")
            for ch in range(4):
                engines[ch % len(engines)].dma_start(
                    t[:, ch * 1024:(ch + 1) * 1024].rearrange(
                        "p (j t) -> p j t", j=2),
                    o_recv[h][ch * 256:(ch + 1) * 256, :].rearrange(
                        "(j p) t -> p j t", p=128),
                )
            return t

        or_big = [None, None]
        load_x([1, 2, 3])
        phase1(0)
        load_x([4, 5, 6, 7])
        for qt in range(S // QT):
            attention(0, 0, qt)
        phase1(1)
        for qt in range(S // QT):
            attention(1, 0, qt)
        a2a(0)  # fires now; transfers while h=1 computes
        or_big[0] = load_or(0)
        for qt in range(S // QT):
            attention(0, 1, qt)
        for qt in range(S // QT):
            attention(1, 1, qt)
        a2a(1)
        or_big[1] = load_or(1)

        # ---- phase 3: out[tok_slice, :] = o.T @ wo.T ---------------------
        # 3a: even-kt partials (h=0 data, available after A2A#0) run during
        # A2A#1; 3b: odd-kt + partial add after A2A#1 lands.
        parts = {}
        for nt in range(4):
            for tb in range(4):
                ps = ps_pv.tile([128, 512], F32, tag="pv")
                for j in range(8):
                    nc.tensor.matmul(
                        ps[:],
                        or_big[0][:, j * 512 + tb * 128: j * 512 + (tb + 1) * 128],
                        wo_t[nt][0][j // 4][:, (j % 4) * 512:(j % 4 + 1) * 512],
                        start=(j == 0),
                        stop=(j == 7),
                    )
                part = part_p.tile([128, 512], BF16, tag="part")
                nc.vector.tensor_copy(part[:], ps[:])
                parts[nt, tb] = part
        for nt in range(4):
            for tb in range(4):
                ps = ps_pv.tile([128, 512], F32, tag="pv")
                for j in range(8):
                    nc.tensor.matmul(
                        ps[:],
                        or_big[1][:, j * 512 + tb * 128: j * 512 + (tb + 1) * 128],
                        wo_t[nt][1][j // 4][:, (j % 4) * 512:(j % 4 + 1) * 512],
                        start=(j == 0),
                        stop=(j == 7),
                    )
                ev = ev_p3.tile([128, 512], F32, tag="ev")
                nc.vector.tensor_add(ev[:], ps[:], parts[nt, tb][:])
                nc.sync.dma_start(
                    out[tb * 128:(tb + 1) * 128, nt * 512:(nt + 1) * 512],
                    ev[:],
                )
    nc.compile()
    return nc


_NC_CACHE = None


def _get_nc():
    global _NC_CACHE
    if _NC_CACHE is None:
        _NC_CACHE = _build()
    return _NC_CACHE


def make_in_maps(x, wq, wk, wv, wo):
    import ml_dtypes

    bf = ml_dtypes.bfloat16
    x = np.asarray(x, dtype=np.float32)
    # tokens b-major: t = b*S + s
    xT = np.ascontiguousarray(x.transpose(2, 1, 0).reshape(H, NT))
    xS = np.ascontiguousarray(
        xT.reshape(KT, 128, 8, NB).transpose(2, 1, 0, 3).reshape(8 * 128, XW)
    ).astype(bf)
    woT = np.asarray(wo, dtype=np.float32).T  # [f_in, f_out]
    woS = np.ascontiguousarray(
        woT.reshape(8, 2, 128, 4, 512).transpose(3, 1, 2, 0, 4).reshape(
            8 * 128, 8 * 512)
    ).astype(bf)

    def wshuf(w, r):
        sl = slice(r * FPC, (r + 1) * FPC)
        wT = np.asarray(w, dtype=np.float32)[sl, :].T  # [H, FPC]
        return np.ascontiguousarray(
            wT.reshape(KT, 128, FPC).transpose(1, 0, 2).reshape(128, KT * FPC)
        ).astype(bf)

    in_maps = []
    for r in range(N_CORES):
        in_maps.append(
            {
                "xS": xS,
                "wqS": wshuf(wq, r),
                "wkS": wshuf(wk, r),
                "wvS": wshuf(wv, r),
                "woS": woS,
            }
        )
    return in_maps


def assemble_out(results):
    out_bs = np.concatenate([results[r]["out"] for r in range(N_CORES)], axis=0)
    return np.ascontiguousarray(out_bs.reshape(B, S, H).transpose(1, 0, 2))


def kernel(x, wq, wk, wv, wo):
    from concourse.bass_utils import run_bass_kernel_spmd

    in_maps = make_in_maps(x, wq, wk, wv, wo)
    res = run_bass_kernel_spmd(_get_nc(), in_maps, list(range(N_CORES)))
    return assemble_out(res.results)


# revision 42
# speedup vs baseline: 1.0638x; 1.0135x over previous
"""Tensor-parallel attention kernel for 8 Trainium2 NeuronCores.

Reference computation (S=2048, B=2, H=2048, NH=16 heads, HD=128):
    q = x @ wq.T ; k = x @ wk.T ; v = x @ wv.T          (x: [S, B, H])
    per (b, head): out = softmax(q k^T / sqrt(HD)) v
    return concat_heads(out) @ wo.T                      ([S, B, H])

Sharding: tensor-parallel over heads (column-parallel wq/wk/wv shards). Core r
owns heads {2r, 2r+1}. The cross-core combine happens BEFORE the output
projection via AllToAll of bf16 attention outputs; each core then applies the
full wo to its 512-token slice.

Schedule (all tokens b-major t = b*S + s):
  phase 1 (per b): qT/kT [256 feat, 2048 tok] = w.T @ x; v [tok, 256] natural
  phase 2, h-outer: all (b, qt) attention units for head 0, then AllToAll #0
      (1 MB) fires while head 1 computes; AllToAll #1 after head 1.
      Softmax denominators: VectorE accumulates the exp tiles (bf16) and a
      single ones-matmul per unit does the partition reduction (replaces 16
      sum-matmuls per unit on the PE).
  phase 3 split by kt parity: even-kt (head-0 senders, delivered by A2A#0)
      partial products run DURING A2A#1; odd-kt + partial add after. Keeps
      the PE busy and HAM-warm through the collective.

DMA: hosts pre-shuffles x/wq/wk/wv/wo into SBUF-tile-order DRAM layouts so
each load is one large DMA with >=1KB contiguous runs (DMA issue on an engine
queue costs ~0.7us each; the baseline spent ~120us of queue time on issues).
"""

import numpy as np

S, B, H = 2048, 2, 2048
NH, HD = 16, 128
N_CORES = 8
HPC = NH // N_CORES          # heads per core (2)
FPC = HPC * HD               # features per core (256)
NT = S * B                   # tokens (4096)
SCALE = HD ** -0.5
KT = H // 128                # contraction tiles (16)
NB = 512                     # token block width in phase 1
XW = KT * NB                 # x big-tile width (8192)
QT = 512                     # q-tile width in phase 2
EXPW = 1024                  # exp batch width (2 key-blocks per ACT op)
JB = S // 128                # key blocks per (b, h) (16)


def _build():
    import concourse.mybir as mybir
    import concourse.tile as tile
    from concourse import bacc

    F32 = mybir.dt.float32
    BF16 = mybir.dt.bfloat16
    Exp = mybir.ActivationFunctionType.Exp

    nc = bacc.Bacc(None, target_bir_lowering=False, num_devices=N_CORES)

    # Pre-shuffled inputs (see make_in_maps):
    #   xS[nb*128+p, kt*NB+t] = x_bf16[feature kt*128+p, token nb*NB+t]
    #   w*S[p, kt*FPC+f]      = w[sl].T[kt*128+p, f]
    #   woS[(nt*2+par)*128+p, j*512+t] = wo.T[(2j+par)*128+p, nt*512+t]
    xS = nc.dram_tensor("xS", [8 * 128, XW], BF16, kind="ExternalInput")
    wqS = nc.dram_tensor("wqS", [128, KT * FPC], BF16, kind="ExternalInput")
    wkS = nc.dram_tensor("wkS", [128, KT * FPC], BF16, kind="ExternalInput")
    wvS = nc.dram_tensor("wvS", [128, KT * FPC], BF16, kind="ExternalInput")
    woS = nc.dram_tensor("woS", [8 * 128, 8 * 512], BF16, kind="ExternalInput")
    out = nc.dram_tensor("out", [NT // N_CORES, H], F32, kind="ExternalOutput")

    from contextlib import ExitStack

    with tile.TileContext(nc) as tc, ExitStack() as ctx:
        pool = lambda **kw: ctx.enter_context(tc.tile_pool(**kw))
        qk_res = pool(name="qk_res", bufs=1)
        v_res = pool(name="v_res", bufs=32)
        const = pool(name="const", bufs=1)
        x0_p = pool(name="x0_p", bufs=2)
        x_p = pool(name="x_p", bufs=2)
        w_p1 = pool(name="w_p1", bufs=1)
        wo_p = pool(name="wo_p", bufs=6)
        p_p2 = pool(name="p_p2", bufs=3)
        acc_p = pool(name="acc_p", bufs=1)
        r_p2 = pool(name="r_p2", bufs=1)
        # 3 bufs: o_send DMA completion (DRAM write) can lag badly when the
        # fabric/HBM is congested; without slack here the pv PSUM pool
        # back-pressures and stalls the PE mid-attention.
        ost_p = pool(name="ost_p", bufs=3)
        or_p = pool(name="or_p", bufs=1)
        part_p = pool(name="part_p", bufs=16)
        ev_p3 = pool(name="ev_p3", bufs=1)
        ps_qk = pool(name="ps_qk", bufs=2, space="PSUM")
        ps_sc = pool(name="ps_sc", bufs=2, space="PSUM")
        ps_pv = pool(name="ps_pv", bufs=2, space="PSUM")
        dram = pool(name="dram", bufs=1, space="DRAM")

        ones_f = const.tile([128, 128], F32)
        nc.vector.memset(ones_f[:], 1.0)
        ones = const.tile([128, 128], BF16)
        nc.vector.tensor_copy(ones[:], ones_f[:])

        qhat = [qk_res.tile([128, NT], BF16, tag=f"q{m}", name=f"qhat{m}")
                for m in range(2)]
        khat = [qk_res.tile([128, NT], BF16, tag=f"k{m}", name=f"khat{m}")
                for m in range(2)]
        vsb = [v_res.tile([128, FPC], BF16, tag="v", name=f"vsb{i}")
               for i in range(NT // 128)]
        o_send = [dram.tile([8 * 128, QT], BF16, name=f"o_send{h}")
                  for h in range(2)]
        o_recv = [dram.tile([8 * 128, QT], BF16, name=f"o_recv{h}")
                  for h in range(2)]

        # ---- input loads -------------------------------------------------
        # weights on the scalar queue; x on the sync queue. All single big
        # DMAs with long contiguous runs thanks to the host pre-shuffle.
        # First q/k group needs wq kt0.. + x0 kt0.. — split the leading loads
        # into halves/quarters across idle queues so the first matmul can
        # start as early as possible after the ~9.5us framework preamble.
        # Startup is DMA-bandwidth-bound: ~5MB (wq+x0+wk+wv) feeds the first
        # ~26us of matmuls. Spread it over three queues (sync, scalar,
        # gpsimd SWDGE) roughly in consumption order.
        wq_all = w_p1.tile([128, KT * FPC], BF16, tag="wq", name="wq_all")
        nc.scalar.dma_start(wq_all[:, 0:KT * FPC // 2], wqS[:, 0:KT * FPC // 2])
        x0a = x0_p.tile([128, XW // 2], BF16, tag="x0a", name="x0a")
        nc.sync.dma_start(x0a[:, 0:XW // 4], xS[0:128, 0:XW // 4])
        nc.scalar.dma_start(wq_all[:, KT * FPC // 2:], wqS[:, KT * FPC // 2:])
        nc.sync.dma_start(x0a[:, XW // 4:], xS[0:128, XW // 4:XW // 2])
        x0b = x0_p.tile([128, XW // 2], BF16, tag="x0b", name="x0b")
        nc.gpsimd.dma_start(x0b[:, 0:XW // 4], xS[0:128, XW // 2:3 * XW // 4])
        nc.gpsimd.dma_start(x0b[:, XW // 4:], xS[0:128, 3 * XW // 4:XW])
        wk_all = w_p1.tile([128, KT * FPC], BF16, tag="wk", name="wk_all")
        nc.scalar.dma_start(wk_all[:, 0:KT * FPC // 2], wkS[:, 0:KT * FPC // 2])
        nc.sync.dma_start(wk_all[:, KT * FPC // 2:], wkS[:, KT * FPC // 2:])
        wv_all = w_p1.tile([128, KT * FPC], BF16, tag="wv", name="wv_all")
        nc.gpsimd.dma_start(wv_all[:], wvS[:, :])

        wq_t = [wq_all[:, kt * FPC:(kt + 1) * FPC] for kt in range(KT)]
        wk_t = [wk_all[:, kt * FPC:(kt + 1) * FPC] for kt in range(KT)]
        wv_t = [wv_all[:, kt * FPC:(kt + 1) * FPC] for kt in range(KT)]

        x_big = {}

        def load_x(nb_list):
            for nb in nb_list:
                t = x_p.tile([128, XW], BF16, tag="x", name=f"x{nb}")
                nc.sync.dma_start(t[:], xS[nb * 128:(nb + 1) * 128, :])
                x_big[nb] = t

        def xt(nb, kt):
            if nb == 0:
                src = x0a if kt < 8 else x0b
                k = kt if kt < 8 else kt - 8
                return src[:, k * NB:(k + 1) * NB]
            return x_big[nb][:, kt * NB:(kt + 1) * NB]

        # wo parity half-tiles: wo_t[nt][par][half][:, jj*512:(jj+1)*512] is
        # the woT tile for contraction block kt=2*(half*4+jj)+par, output
        # columns nt*512... Loaded in 4KB/partition chunks through a 4-buffer
        # pool so phase-3 streaming never stalls on a 1MB transfer.
        # par1/half1 tiles are deferred: they load on the sync queue during
        # phase 3 (emitted after the or1 chunks), halving the scalar queue's
        # phase-3 wo streaming load. gpsimd SWDGE measured slower than the
        # HWDGE queues and starves the 3b stream — scalar/sync only.
        wo_t = [[[None, None], [None, None]] for _ in range(4)]

        def load_wo(nt, par, half, eng):
            t = wo_p.tile([128, 4 * 512], BF16, tag="wo",
                          name=f"wo{nt}_{par}_{half}")
            r0 = (nt * 2 + par) * 128
            eng.dma_start(
                t[:], woS[r0:r0 + 128, half * 2048:(half + 1) * 2048]
            )
            wo_t[nt][par][half] = t

        for par in range(2):
            for nt in range(4):
                for half in range(2):
                    if par == 1 and half == 1:
                        continue
                    load_wo(nt, par, half, nc.scalar)

        # ---- phase 1: qT/kT/v projections --------------------------------
        def phase1(b):
            for nb in range(b * S // NB, (b + 1) * S // NB):
                for dest, wt in ((qhat, wq_t), (khat, wk_t)):
                    for m in range(2):
                        ps = ps_qk.tile([128, NB], F32, tag="qk")
                        for kt in range(KT):
                            nc.tensor.matmul(
                                ps[:],
                                wt[kt][:, m * 128:(m + 1) * 128],
                                xt(nb, kt),
                                start=(kt == 0),
                                stop=(kt == KT - 1),
                            )
                        nc.vector.tensor_copy(
                            dest[m][:, nb * NB:(nb + 1) * NB], ps[:]
                        )
                for sub in range(NB // 128):
                    ps = ps_qk.tile([128, FPC], F32, tag="qk")
                    for kt in range(KT):
                        nc.tensor.matmul(
                            ps[:],
                            xt(nb, kt)[:, sub * 128:(sub + 1) * 128],
                            wv_t[kt][:],
                            start=(kt == 0),
                            stop=(kt == KT - 1),
                        )
                    nc.vector.tensor_copy(vsb[nb * 4 + sub][:], ps[:])

        # ---- phase 2: attention for one (b, h, qt) unit ------------------
        def attention(b, h, qt):
            q_bh = qhat[h][:, b * S + qt * QT: b * S + (qt + 1) * QT]
            pv_ps = ps_pv.tile([128, QT], F32, tag="pv")
            acc = acc_p.tile([128, QT], BF16, tag="acc")
            for g in range(JB // 2):
                sc_ps = ps_sc.tile([128, EXPW], F32, tag="sc")
                pT = p_p2.tile([128, EXPW], BF16, tag="p")
                for i in range(2):
                    jb = g * 2 + i
                    nc.tensor.matmul(
                        sc_ps[:, i * QT:(i + 1) * QT],
                        khat[h][:, b * S + jb * 128: b * S + (jb + 1) * 128],
                        q_bh,
                        start=True,
                        stop=True,
                    )
                nc.scalar.activation(pT[:], sc_ps[:], Exp, scale=SCALE)
                if g == 0:
                    nc.vector.tensor_add(acc[:], pT[:, 0:QT], pT[:, QT:EXPW])
                else:
                    nc.vector.tensor_add(acc[:], acc[:], pT[:, 0:QT])
                    nc.vector.tensor_add(acc[:], acc[:], pT[:, QT:EXPW])
                for i in range(2):
                    jb = g * 2 + i
                    nc.tensor.matmul(
                        pv_ps[:],
                        vsb[b * JB + jb][:, h * 128:(h + 1) * 128],
                        pT[:, i * QT:(i + 1) * QT],
                        start=(jb == 0),
                        stop=(jb == JB - 1),
                    )
            sum_ps = ps_qk.tile([128, QT], F32, tag="qk")
            nc.tensor.matmul(sum_ps[:], ones[:], acc[:], start=True, stop=True)
            recip = r_p2.tile([128, QT], F32, tag="r")
            nc.vector.reciprocal_approx_fast(recip[:], sum_ps[:])
            ostg = ost_p.tile([128, QT], BF16, tag="ost")
            nc.vector.tensor_mul(ostg[:], pv_ps[:], recip[:])
            c = b * (S // QT) + qt
            nc.sync.dma_start(o_send[h][c * 128:(c + 1) * 128, :], ostg[:])

        # ---- main schedule ----------------------------------------------
        def a2a(h):
            # gpsimd reaches the trigger early and fires as soon as the
            # o_send[h] writes land; compute engines keep running.
            nc.gpsimd.collective_compute(
                "AllToAll",
                mybir.AluOpType.bypass,
                replica_groups=[list(range(N_CORES))],
                ins=[o_send[h][:].opt()],
                outs=[o_recv[h][:].opt()],
            )

        def load_or(h):
            # On the gpsimd queue, which is idle apart from the collective
            # triggers: the load starts the instant the A2A completes, and
            # never head-of-line-blocks a busy queue (a sync-queue DMA
            # waiting on A2A completion ahead of pending o_send writes would
            # back-pressure the attention pipeline via the pv PSUM pool).
            # Four chunks so phase 3 can start on the first senders' blocks
            # while the rest stream in.
            # or0 rides gpsimd (slack: it loads mid-h1; sync still has h=1
            # o_send writes pending and must not be blocked). or1 is on the
            # phase-3 critical path: stripe it over the two fast HWDGE
            # queues, both idle at that point.
            engines = [nc.gpsimd] if h == 0 else [nc.sync, nc.scalar]
            t = or_p.tile([128, 8 * 512], BF16, tag=f"or{h}", name=f"or{h}")
            for ch in range(4):
                engines[ch % len(engines)].dma_start(
                    t[:, ch * 1024:(ch + 1) * 1024].rearrange(
                        "p (j t) -> p j t", j=2),
                    o_recv[h][ch * 256:(ch + 1) * 256, :].rearrange(
                        "(j p) t -> p j t", p=128),
                )
            return t

        or_big = [None, None]
        load_x([1, 2, 3])
        phase1(0)
        load_x([4, 5, 6, 7])
        for qt in range(S // QT):
            attention(0, 0, qt)
        phase1(1)
        for qt in range(S // QT):
            attention(1, 0, qt)
        a2a(0)  # fires now; transfers while h=1 computes
        or_big[0] = load_or(0)
        for qt in range(S // QT):
            attention(0, 1, qt)
        for qt in range(S // QT):
            attention(1, 1, qt)
        a2a(1)
        or_big[1] = load_or(1)
        # deferred wo par1/half1 loads: queued on sync behind the or1 chunks
        # (ahead only of the out-writes, which depend on 3b anyway)
        for nt in range(4):
            load_wo(nt, 1, 1, nc.sync)

        # ---- phase 3: out[tok_slice, :] = o.T @ wo.T ---------------------
        # 3a: even-kt partials (h=0 data, available after A2A#0) run during
        # A2A#1; 3b: odd-kt + partial add after A2A#1 lands.
        parts = {}
        for nt in range(4):
            for tb in range(4):
                ps = ps_pv.tile([128, 512], F32, tag="pv")
                for j in range(8):
                    nc.tensor.matmul(
                        ps[:],
                        or_big[0][:, j * 512 + tb * 128: j * 512 + (tb + 1) * 128],
                        wo_t[nt][0][j // 4][:, (j % 4) * 512:(j % 4 + 1) * 512],
                        start=(j == 0),
                        stop=(j == 7),
                    )
                part = part_p.tile([128, 512], BF16, tag="part")
                nc.vector.tensor_copy(part[:], ps[:])
                parts[nt, tb] = part
        for nt in range(4):
            for tb in range(4):
                ps = ps_pv.tile([128, 512], F32, tag="pv")
                for j in range(8):
                    nc.tensor.matmul(
                        ps[:],
                        or_big[1][:, j * 512 + tb * 128: j * 512 + (tb + 1) * 128],
                        wo_t[nt][1][j // 4][:, (j % 4) * 512:(j % 4 + 1) * 512],
                        start=(j == 0),
                        stop=(j == 7),
                    )
                ev = ev_p3.tile([128, 512], F32, tag="ev")
                nc.vector.tensor_add(ev[:], ps[:], parts[nt, tb][:])
                nc.sync.dma_start(
                    out[tb * 128:(tb + 1) * 128, nt * 512:(nt + 1) * 512],
                    ev[:],
                )
    nc.compile()
    return nc


_NC_CACHE = None


def _get_nc():
    global _NC_CACHE
    if _NC_CACHE is None:
        _NC_CACHE = _build()
    return _NC_CACHE


def make_in_maps(x, wq, wk, wv, wo):
    import ml_dtypes

    bf = ml_dtypes.bfloat16
    x = np.asarray(x, dtype=np.float32)
    # tokens b-major: t = b*S + s
    xT = np.ascontiguousarray(x.transpose(2, 1, 0).reshape(H, NT))
    xS = np.ascontiguousarray(
        xT.reshape(KT, 128, 8, NB).transpose(2, 1, 0, 3).reshape(8 * 128, XW)
    ).astype(bf)
    woT = np.asarray(wo, dtype=np.float32).T  # [f_in, f_out]
    woS = np.ascontiguousarray(
        woT.reshape(8, 2, 128, 4, 512).transpose(3, 1, 2, 0, 4).reshape(
            8 * 128, 8 * 512)
    ).astype(bf)

    def wshuf(w, r):
        sl = slice(r * FPC, (r + 1) * FPC)
        wT = np.asarray(w, dtype=np.float32)[sl, :].T  # [H, FPC]
        return np.ascontiguousarray(
            wT.reshape(KT, 128, FPC).transpose(1, 0, 2).reshape(128, KT * FPC)
        ).astype(bf)

    in_maps = []
    for r in range(N_CORES):
        in_maps.append(
            {
                "xS": xS,
                "wqS": wshuf(wq, r),
                "wkS": wshuf(wk, r),
                "wvS": wshuf(wv, r),
                "woS": woS,
            }
        )
    return in_maps


def assemble_out(results):
    out_bs = np.concatenate([results[r]["out"] for r in range(N_CORES)], axis=0)
    return np.ascontiguousarray(out_bs.reshape(B, S, H).transpose(1, 0, 2))


def kernel(x, wq, wk, wv, wo):
    from concourse.bass_utils import run_bass_kernel_spmd

    in_maps = make_in_maps(x, wq, wk, wv, wo)
    res = run_bass_kernel_spmd(_get_nc(), in_maps, list(range(N_CORES)))
    return assemble_out(res.results)


# revision 44
# speedup vs baseline: 1.1146x; 1.0478x over previous
"""Tensor-parallel attention kernel for 8 Trainium2 NeuronCores.

Reference computation (S=2048, B=2, H=2048, NH=16 heads, HD=128):
    q = x @ wq.T ; k = x @ wk.T ; v = x @ wv.T          (x: [S, B, H])
    per (b, head): out = softmax(q k^T / sqrt(HD)) v
    return concat_heads(out) @ wo.T                      ([S, B, H])

Sharding: tensor-parallel over heads (column-parallel wq/wk/wv shards). Core r
owns heads {2r, 2r+1}. The cross-core combine happens BEFORE the output
projection via AllToAll of bf16 attention outputs; each core then applies the
full wo to its 512-token slice.

Schedule (all tokens b-major t = b*S + s):
  phase 1 (per b): qT/kT [256 feat, 2048 tok] = w.T @ x; v [tok, 256] natural
  phase 2, h-outer: all (b, qt) attention units for head 0, then AllToAll #0
      (1 MB) fires while head 1 computes; AllToAll #1 after head 1.
      Softmax denominators: VectorE accumulates the exp tiles (bf16) and a
      single ones-matmul per unit does the partition reduction (replaces 16
      sum-matmuls per unit on the PE).
  phase 3 split by kt parity: even-kt (head-0 senders, delivered by A2A#0)
      partial products run DURING A2A#1; odd-kt + partial add after. Keeps
      the PE busy and HAM-warm through the collective.

DMA: hosts pre-shuffles x/wq/wk/wv/wo into SBUF-tile-order DRAM layouts so
each load is one large DMA with >=1KB contiguous runs (DMA issue on an engine
queue costs ~0.7us each; the baseline spent ~120us of queue time on issues).
"""

import numpy as np

S, B, H = 2048, 2, 2048
NH, HD = 16, 128
N_CORES = 8
HPC = NH // N_CORES          # heads per core (2)
FPC = HPC * HD               # features per core (256)
NT = S * B                   # tokens (4096)
SCALE = HD ** -0.5
KT = H // 128                # contraction tiles (16)
NB = 512                     # token block width in phase 1
XW = KT * NB                 # x big-tile width (8192)
QT = 512                     # q-tile width in phase 2
EXPW = 1024                  # exp batch width (2 key-blocks per ACT op)
JB = S // 128                # key blocks per (b, h) (16)


def _build():
    import concourse.mybir as mybir
    import concourse.tile as tile
    from concourse import bacc

    F32 = mybir.dt.float32
    BF16 = mybir.dt.bfloat16
    Exp = mybir.ActivationFunctionType.Exp

    nc = bacc.Bacc(None, target_bir_lowering=False, num_devices=N_CORES)

    # Pre-shuffled inputs (see make_in_maps):
    #   xS[nb*128+p, kt*NB+t] = x_bf16[feature kt*128+p, token nb*NB+t]
    #   w*S[p, kt*FPC+f]      = w[sl].T[kt*128+p, f]
    #   woS[(nt*2+par)*128+p, j*512+t] = wo.T[(2j+par)*128+p, nt*512+t]
    xS = nc.dram_tensor("xS", [8 * 128, XW], BF16, kind="ExternalInput")
    wqS = nc.dram_tensor("wqS", [128, KT * FPC], BF16, kind="ExternalInput")
    wkS = nc.dram_tensor("wkS", [128, KT * FPC], BF16, kind="ExternalInput")
    wvS = nc.dram_tensor("wvS", [128, KT * FPC], BF16, kind="ExternalInput")
    woS = nc.dram_tensor("woS", [8 * 128, 8 * 512], BF16, kind="ExternalInput")
    out = nc.dram_tensor("out", [NT // N_CORES, H], F32, kind="ExternalOutput")

    from contextlib import ExitStack

    with tile.TileContext(nc) as tc, ExitStack() as ctx:
        pool = lambda **kw: ctx.enter_context(tc.tile_pool(**kw))
        qk_res = pool(name="qk_res", bufs=1)
        v_res = pool(name="v_res", bufs=32)
        const = pool(name="const", bufs=1)
        x0_p = pool(name="x0_p", bufs=2)
        x_p = pool(name="x_p", bufs=2)
        w_p1 = pool(name="w_p1", bufs=1)
        wo_p = pool(name="wo_p", bufs=6)
        p_p2 = pool(name="p_p2", bufs=2)
        acc_p = pool(name="acc_p", bufs=1)
        r_p2 = pool(name="r_p2", bufs=1)
        # 3 bufs: o_send DMA completion (DRAM write) can lag badly when the
        # fabric/HBM is congested; without slack here the pv PSUM pool
        # back-pressures and stalls the PE mid-attention.
        ost_p = pool(name="ost_p", bufs=3)
        or_p = pool(name="or_p", bufs=1)
        part_p = pool(name="part_p", bufs=16)
        ev_p3 = pool(name="ev_p3", bufs=2)
        ps_qk = pool(name="ps_qk", bufs=2, space="PSUM")
        ps_sc = pool(name="ps_sc", bufs=2, space="PSUM")
        ps_pv = pool(name="ps_pv", bufs=2, space="PSUM")
        dram = pool(name="dram", bufs=1, space="DRAM")

        ones_f = const.tile([128, 128], F32)
        nc.vector.memset(ones_f[:], 1.0)
        ones = const.tile([128, 128], BF16)
        nc.vector.tensor_copy(ones[:], ones_f[:])

        qhat = [qk_res.tile([128, NT], BF16, tag=f"q{m}", name=f"qhat{m}")
                for m in range(2)]
        khat = [qk_res.tile([128, NT], BF16, tag=f"k{m}", name=f"khat{m}")
                for m in range(2)]
        vsb = [v_res.tile([128, FPC], BF16, tag="v", name=f"vsb{i}")
               for i in range(NT // 128)]
        o_send = [dram.tile([8 * 128, QT], BF16, name=f"o_send{h}")
                  for h in range(2)]
        o_recv = [dram.tile([8 * 128, QT], BF16, name=f"o_recv{h}")
                  for h in range(2)]

        # ---- input loads -------------------------------------------------
        # weights on the scalar queue; x on the sync queue. All single big
        # DMAs with long contiguous runs thanks to the host pre-shuffle.
        # First q/k group needs wq kt0.. + x0 kt0.. — split the leading loads
        # into halves/quarters across idle queues so the first matmul can
        # start as early as possible after the ~9.5us framework preamble.
        # Startup is DMA-bandwidth-bound: ~5MB (wq+x0+wk+wv) feeds the first
        # ~26us of matmuls. Spread it over three queues (sync, scalar,
        # gpsimd SWDGE) roughly in consumption order.
        wq_all = w_p1.tile([128, KT * FPC], BF16, tag="wq", name="wq_all")
        nc.scalar.dma_start(wq_all[:, 0:KT * FPC // 2], wqS[:, 0:KT * FPC // 2])
        x0a = x0_p.tile([128, XW // 2], BF16, tag="x0a", name="x0a")
        nc.sync.dma_start(x0a[:, 0:XW // 4], xS[0:128, 0:XW // 4])
        nc.scalar.dma_start(wq_all[:, KT * FPC // 2:], wqS[:, KT * FPC // 2:])
        nc.sync.dma_start(x0a[:, XW // 4:], xS[0:128, XW // 4:XW // 2])
        x0b = x0_p.tile([128, XW // 2], BF16, tag="x0b", name="x0b")
        nc.gpsimd.dma_start(x0b[:, 0:XW // 4], xS[0:128, XW // 2:3 * XW // 4])
        nc.gpsimd.dma_start(x0b[:, XW // 4:], xS[0:128, 3 * XW // 4:XW])
        wk_all = w_p1.tile([128, KT * FPC], BF16, tag="wk", name="wk_all")
        nc.scalar.dma_start(wk_all[:, 0:KT * FPC // 2], wkS[:, 0:KT * FPC // 2])
        nc.sync.dma_start(wk_all[:, KT * FPC // 2:], wkS[:, KT * FPC // 2:])
        wv_all = w_p1.tile([128, KT * FPC], BF16, tag="wv", name="wv_all")
        nc.gpsimd.dma_start(wv_all[:], wvS[:, :])

        wq_t = [wq_all[:, kt * FPC:(kt + 1) * FPC] for kt in range(KT)]
        wk_t = [wk_all[:, kt * FPC:(kt + 1) * FPC] for kt in range(KT)]
        wv_t = [wv_all[:, kt * FPC:(kt + 1) * FPC] for kt in range(KT)]

        x_big = {}

        def load_x(nb_list):
            for nb in nb_list:
                t = x_p.tile([128, XW], BF16, tag="x", name=f"x{nb}")
                nc.sync.dma_start(t[:], xS[nb * 128:(nb + 1) * 128, :])
                x_big[nb] = t

        def xt(nb, kt):
            if nb == 0:
                src = x0a if kt < 8 else x0b
                k = kt if kt < 8 else kt - 8
                return src[:, k * NB:(k + 1) * NB]
            return x_big[nb][:, kt * NB:(kt + 1) * NB]

        # wo parity half-tiles: wo_t[nt][par][half][:, jj*512:(jj+1)*512] is
        # the woT tile for contraction block kt=2*(half*4+jj)+par, output
        # columns nt*512... Loaded in 4KB/partition chunks through a 4-buffer
        # pool so phase-3 streaming never stalls on a 1MB transfer.
        # par1/half1 tiles are deferred: they load on the sync queue during
        # phase 3 (emitted after the or1 chunks), halving the scalar queue's
        # phase-3 wo streaming load. gpsimd SWDGE measured slower than the
        # HWDGE queues and starves the 3b stream — scalar/sync only.
        wo_t = [[[None, None], [None, None]] for _ in range(4)]

        def load_wo(nt, par, half, eng):
            t = wo_p.tile([128, 4 * 512], BF16, tag="wo",
                          name=f"wo{nt}_{par}_{half}")
            r0 = (nt * 2 + par) * 128
            eng.dma_start(
                t[:], woS[r0:r0 + 128, half * 2048:(half + 1) * 2048]
            )
            wo_t[nt][par][half] = t

        for par in range(2):
            for nt in range(4):
                for half in range(2):
                    if par == 1 and half == 1:
                        continue
                    load_wo(nt, par, half, nc.scalar)

        # ---- phase 1: qT/kT/v projections --------------------------------
        def phase1(b):
            for nb in range(b * S // NB, (b + 1) * S // NB):
                for dest, wt in ((qhat, wq_t), (khat, wk_t)):
                    for m in range(2):
                        ps = ps_qk.tile([128, NB], F32, tag="qk")
                        for kt in range(KT):
                            nc.tensor.matmul(
                                ps[:],
                                wt[kt][:, m * 128:(m + 1) * 128],
                                xt(nb, kt),
                                start=(kt == 0),
                                stop=(kt == KT - 1),
                            )
                        nc.vector.tensor_copy(
                            dest[m][:, nb * NB:(nb + 1) * NB], ps[:]
                        )
                for sub in range(NB // 128):
                    ps = ps_qk.tile([128, FPC], F32, tag="qk")
                    for kt in range(KT):
                        nc.tensor.matmul(
                            ps[:],
                            xt(nb, kt)[:, sub * 128:(sub + 1) * 128],
                            wv_t[kt][:],
                            start=(kt == 0),
                            stop=(kt == KT - 1),
                        )
                    nc.vector.tensor_copy(vsb[nb * 4 + sub][:], ps[:])

        # ---- phase 2: attention for one (b, h, qt) unit ------------------
        def attention(b, h, qt):
            q_bh = qhat[h][:, b * S + qt * QT: b * S + (qt + 1) * QT]
            pv_ps = ps_pv.tile([128, QT], F32, tag="pv")
            acc = acc_p.tile([128, QT], BF16, tag="acc")
            for g in range(JB // 2):
                sc_ps = ps_sc.tile([128, EXPW], F32, tag="sc")
                pT = p_p2.tile([128, EXPW], BF16, tag="p")
                for i in range(2):
                    jb = g * 2 + i
                    nc.tensor.matmul(
                        sc_ps[:, i * QT:(i + 1) * QT],
                        khat[h][:, b * S + jb * 128: b * S + (jb + 1) * 128],
                        q_bh,
                        start=True,
                        stop=True,
                    )
                nc.scalar.activation(pT[:], sc_ps[:], Exp, scale=SCALE)
                if g == 0:
                    nc.vector.tensor_add(acc[:], pT[:, 0:QT], pT[:, QT:EXPW])
                else:
                    nc.vector.tensor_add(acc[:], acc[:], pT[:, 0:QT])
                    nc.vector.tensor_add(acc[:], acc[:], pT[:, QT:EXPW])
                for i in range(2):
                    jb = g * 2 + i
                    nc.tensor.matmul(
                        pv_ps[:],
                        vsb[b * JB + jb][:, h * 128:(h + 1) * 128],
                        pT[:, i * QT:(i + 1) * QT],
                        start=(jb == 0),
                        stop=(jb == JB - 1),
                    )
            sum_ps = ps_qk.tile([128, QT], F32, tag="qk")
            nc.tensor.matmul(sum_ps[:], ones[:], acc[:], start=True, stop=True)
            recip = r_p2.tile([128, QT], F32, tag="r")
            nc.vector.reciprocal_approx_fast(recip[:], sum_ps[:])
            ostg = ost_p.tile([128, QT], BF16, tag="ost")
            nc.vector.tensor_mul(ostg[:], pv_ps[:], recip[:])
            c = b * (S // QT) + qt
            nc.sync.dma_start(o_send[h][c * 128:(c + 1) * 128, :], ostg[:])

        # ---- main schedule ----------------------------------------------
        def a2a(h):
            # gpsimd reaches the trigger early and fires as soon as the
            # o_send[h] writes land; compute engines keep running.
            nc.gpsimd.collective_compute(
                "AllToAll",
                mybir.AluOpType.bypass,
                replica_groups=[list(range(N_CORES))],
                ins=[o_send[h][:].opt()],
                outs=[o_recv[h][:].opt()],
            )

        def load_or(h):
            # On the gpsimd queue, which is idle apart from the collective
            # triggers: the load starts the instant the A2A completes, and
            # never head-of-line-blocks a busy queue (a sync-queue DMA
            # waiting on A2A completion ahead of pending o_send writes would
            # back-pressure the attention pipeline via the pv PSUM pool).
            # Four chunks so phase 3 can start on the first senders' blocks
            # while the rest stream in.
            # or0 rides gpsimd (slack: it loads mid-h1; sync still has h=1
            # o_send writes pending and must not be blocked). or1 is on the
            # phase-3 critical path: stripe it over the two fast HWDGE
            # queues, both idle at that point.
            engines = [nc.gpsimd] if h == 0 else [nc.sync, nc.scalar]
            t = or_p.tile([128, 8 * 512], BF16, tag=f"or{h}", name=f"or{h}")
            for ch in range(4):
                engines[ch % len(engines)].dma_start(
                    t[:, ch * 1024:(ch + 1) * 1024].rearrange(
                        "p (j t) -> p j t", j=2),
                    o_recv[h][ch * 256:(ch + 1) * 256, :].rearrange(
                        "(j p) t -> p j t", p=128),
                )
            return t

        or_big = [None, None]
        load_x([1, 2, 3])
        phase1(0)
        load_x([4, 5, 6, 7])
        for qt in range(S // QT):
            attention(0, 0, qt)
        phase1(1)
        for qt in range(S // QT):
            attention(1, 0, qt)
        a2a(0)  # fires now; transfers while h=1 computes
        or_big[0] = load_or(0)
        for qt in range(S // QT):
            attention(0, 1, qt)
        for qt in range(S // QT):
            attention(1, 1, qt)
        a2a(1)
        or_big[1] = load_or(1)
        # deferred wo par1/half1 loads: queued on sync behind the or1 chunks
        # (ahead only of the out-writes, which depend on 3b anyway)
        for nt in range(4):
            load_wo(nt, 1, 1, nc.sync)

        # ---- phase 3: out[tok_slice, :] = o.T @ wo.T ---------------------
        # 3a: even-kt partials (h=0 data, available after A2A#0) run during
        # A2A#1; 3b: odd-kt + partial add after A2A#1 lands.
        parts = {}
        for nt in range(4):
            for tb in range(4):
                ps = ps_pv.tile([128, 512], F32, tag="pv")
                for j in range(8):
                    nc.tensor.matmul(
                        ps[:],
                        or_big[0][:, j * 512 + tb * 128: j * 512 + (tb + 1) * 128],
                        wo_t[nt][0][j // 4][:, (j % 4) * 512:(j % 4 + 1) * 512],
                        start=(j == 0),
                        stop=(j == 7),
                    )
                part = part_p.tile([128, 512], BF16, tag="part")
                nc.vector.tensor_copy(part[:], ps[:])
                parts[nt, tb] = part
        for nt in range(4):
            for tb in range(4):
                ps = ps_pv.tile([128, 512], F32, tag="pv")
                for j in range(8):
                    nc.tensor.matmul(
                        ps[:],
                        or_big[1][:, j * 512 + tb * 128: j * 512 + (tb + 1) * 128],
                        wo_t[nt][1][j // 4][:, (j % 4) * 512:(j % 4 + 1) * 512],
                        start=(j == 0),
                        stop=(j == 7),
                    )
                ev = ev_p3.tile([128, 512], F32, tag="ev")
                nc.vector.tensor_add(ev[:], ps[:], parts[nt, tb][:])
                nc.sync.dma_start(
                    out[tb * 128:(tb + 1) * 128, nt * 512:(nt + 1) * 512],
                    ev[:],
                )
    nc.compile()
    return nc


_NC_CACHE = None


def _get_nc():
    global _NC_CACHE
    if _NC_CACHE is None:
        _NC_CACHE = _build()
    return _NC_CACHE


def make_in_maps(x, wq, wk, wv, wo):
    import ml_dtypes

    bf = ml_dtypes.bfloat16
    x = np.asarray(x, dtype=np.float32)
    # tokens b-major: t = b*S + s
    xT = np.ascontiguousarray(x.transpose(2, 1, 0).reshape(H, NT))
    xS = np.ascontiguousarray(
        xT.reshape(KT, 128, 8, NB).transpose(2, 1, 0, 3).reshape(8 * 128, XW)
    ).astype(bf)
    woT = np.asarray(wo, dtype=np.float32).T  # [f_in, f_out]
    woS = np.ascontiguousarray(
        woT.reshape(8, 2, 128, 4, 512).transpose(3, 1, 2, 0, 4).reshape(
            8 * 128, 8 * 512)
    ).astype(bf)

    def wshuf(w, r):
        sl = slice(r * FPC, (r + 1) * FPC)
        wT = np.asarray(w, dtype=np.float32)[sl, :].T  # [H, FPC]
        return np.ascontiguousarray(
            wT.reshape(KT, 128, FPC).transpose(1, 0, 2).reshape(128, KT * FPC)
        ).astype(bf)

    in_maps = []
    for r in range(N_CORES):
        in_maps.append(
            {
                "xS": xS,
                "wqS": wshuf(wq, r),
                "wkS": wshuf(wk, r),
                "wvS": wshuf(wv, r),
                "woS": woS,
            }
        )
    return in_maps


def assemble_out(results):
    out_bs = np.concatenate([results[r]["out"] for r in range(N_CORES)], axis=0)
    return np.ascontiguousarray(out_bs.reshape(B, S, H).transpose(1, 0, 2))


def kernel(x, wq, wk, wv, wo):
    from concourse.bass_utils import run_bass_kernel_spmd

    in_maps = make_in_maps(x, wq, wk, wv, wo)
    res = run_bass_kernel_spmd(_get_nc(), in_maps, list(range(N_CORES)))
    return assemble_out(res.results)
